# revision 64
# baseline (speedup 1.0000x reference)
"""Causal single-head attention layer on 8 TRN2 NeuronCores.

Reference (per batch b):
  Q = x@Wq+bq; K = x@Wk+bk; V = x@Wv+bv        (S=4096, D=512, H=64)
  S = Q K^T / sqrt(S);  P = softmax(S + causal_mask);  out = (P V) @ Wo + bo

Sharding: 8 cores = 4 batches x 2 halves. Each core owns 4 query-blocks
of 512 rows of its batch in ASCENDING causal order: even cores take
blocks [0,3,4,7], odd take [1,2,5,6]. SPMD structural k-tile counts per
slot NKT=[8,16,24,32] cover both parities; over-structural k-tiles and
the causal boundary are killed by an additive -1e5 mask generated
ON-CHIP from an iota ramp compared against a per-core threshold input
(thr[s] = (NKT[s]-8)*128 - 512*blk_s) -- no mask DMA.

Schedule (per-engine instruction streams execute in emission order):
KV-projection chunks, Q-projection, attention slots and epilogues are
interleaved so slot s runs as soon as k-tiles 0..NKT[s]-1 exist;
ascending slot order makes slot 0 ready after 1/4 of the projection.
Hard-won scheduling rules baked in here:
 - a dma_start BLOCKS its issuing engine until the DMA's input data is
   ready, and each DMA queue is FIFO -- so every load DMA is emitted
   before any DMA that depends on on-chip compute (V^T XBAR
   transposes, output tiles), and output DMAs for slots 0-2 are
   deferred to the pre-slot-3 bunch;
 - the gpsimd SWDGE queue stalls while the gpsimd engine computes, so
   all xt bulk goes over the sync HWDGE queue (weights over scalar's);
 - the PE downclocks when idle and takes ~2.5us of heavy activity to
   reach full speed, so dependency-free warm-up matmuls burn the
   ~13us DMA lead-in and keep the ramp alive through early data gaps;
 - epilogue recip/scale work is bunched right before slot 3, whose
   first 12 groups need no vector work, so it can never stall an AV.
Engine split: scalar = exp ACTs (+slot-3 tail); sync = xt/out DMA;
gpsimd = xtq DMA + iota; vector = bias/scale/mask work.

Per group of 2 k-tiles: S^T [128k,1024] = K^T.T @ Q^T (bf16 PE, K^T
read straight from kvt[64:128] with Q^T placed at base partition 64)
-> exp via ACT (scale=1/64 folded) -> P^T bf16 -> multiplicative
{0,1} mask (tail groups only, DVE) -> AV accumulate otp[65,512]
(V_aug carries a ones column so the softmax denominator falls out of
row 64; V natural layout produced by XBAR transpose DMAs). QK/AV are
software-pipelined (QK(g+1) before AV(g)) so the PE does not wait on
the exp. Epilogue: denominator row -> partitions via K=1 matmul,
reciprocal, y = ot^T @ [Wo; bv@Wo+bo] scaled by 1/denom -> bf16 out
DMA (host casts to f32). Softmax max-subtraction skipped: |S/64| <~ 1
so exp is safe.
"""

import os

os.environ.setdefault("MYCRO_LOCAL_CACHE", "1")

import numpy as np
import ml_dtypes

import concourse.mybir as mybir
import concourse.tile as tile
from concourse import bacc
from concourse.bass_utils import run_bass_kernel_spmd

F32 = mybir.dt.float32
BF16 = mybir.dt.bfloat16

B, S, D, H = 4, 4096, 512, 64
QB = 512
NKT = [8, 16, 24, 32]          # structural k-tiles per slot (ascending)
BLOCKS_EVEN = [0, 3, 4, 7]
BLOCKS_ODD = [1, 2, 5, 6]

LAST_EXEC_TIME_NS = None
LAST_RESULTS = None


def _install_ntff_hook():
    import sys
    import types
    try:
        from antenv.axon_hooks import get_axon_ntff_profile_hook  # noqa: F401
        return True
    except ImportError:
        pass
    try:
        import trn_agent_boot.trn_boot as _tb
        hook = _tb._ntff_profile_via_ctypes("/opt/axon/libaxon_pjrt.so")
        if hook is None:
            return False
        mod = types.ModuleType("antenv.axon_hooks")
        mod.get_axon_ntff_profile_hook = lambda: hook
        mod.set_axon_ntff_profile_hook = lambda h: None
        sys.modules["antenv.axon_hooks"] = mod
        return True
    except Exception:
        return False


def _build_nc():
    nc = bacc.Bacc(
        "TRN2",
        target_bir_lowering=False,
        debug=False,
        enable_asserts=False,
        num_devices=8,
    )

    xt_d = nc.dram_tensor("xt", [D, S], BF16, kind="ExternalInput")
    xtq_d = nc.dram_tensor("xtq", [D, 4 * QB], BF16, kind="ExternalInput")
    wpack_d = nc.dram_tensor("wpack", [128, 768], BF16, kind="ExternalInput")
    wo_d = nc.dram_tensor("wo", [H + 1, D], BF16, kind="ExternalInput")
    fpack_d = nc.dram_tensor("fpack", [128, 6], F32, kind="ExternalInput")
    out_d = nc.dram_tensor("out", [4 * QB, D], BF16, kind="ExternalOutput")

    with tile.TileContext(nc) as tc:
        with (
            tc.tile_pool(name="big", bufs=1) as big,
            tc.tile_pool(name="small", bufs=1) as small,
            tc.tile_pool(name="projps", bufs=2, space="PSUM") as projps,
            tc.tile_pool(name="stps", bufs=2, space="PSUM") as stps,
            tc.tile_pool(name="otps", bufs=2, space="PSUM") as otps,
            tc.tile_pool(name="ptp", bufs=4) as ptp,
            tc.tile_pool(name="epi", bufs=2) as epi,
        ):
            # ---- persistent SBUF ----
            xt_sb = [big.tile([128, S], BF16, name=f"xt{j}", tag=f"xt{j}") for j in range(4)]
            xtq_sb = [big.tile([128, 4 * QB], BF16, name=f"xtq{j}", tag=f"xtq{j}") for j in range(4)]
            kvt_sb = big.tile([128, S], BF16, tag="kvt")      # rows 0:64 V^T, 64:128 K^T
            qtp_sb = big.tile([128, 4 * QB], BF16, tag="qtp")  # Q^T on partitions 64:128
            vaug_sb = big.tile([128, 32 * 80], BF16, tag="vaug")
            iota_sb = big.tile([128, 8, 512], F32, tag="iota")
            mask_sb = big.tile([128, 4, 4096], BF16, tag="mask")
            wpack_sb = small.tile([128, 768], BF16, tag="wpack")
            wo_sb = small.tile([H + 1, D], BF16, tag="wo")
            fpack_sb = small.tile([128, 6], F32, tag="fpack")
            ones_sb = small.tile([1, 1], BF16, tag="ones")

            vaug3 = vaug_sb[:].rearrange("p (k c) -> p k c", c=80)
            bkv_ap = fpack_sb[:, 0:1]
            bq_ap = fpack_sb[0:64, 1:2]

            # ---- input DMAs ----
            # scalar: packed weights only (its queue is idle until the
            # first exp ACT; keeps sync/gpsimd free for bulk x)
            nc.scalar.dma_start(out=wpack_sb[:], in_=wpack_d[:, :])
            nc.scalar.dma_start(out=fpack_sb[:], in_=fpack_d[:, :])
            nc.scalar.dma_start(out=wo_sb[:], in_=wo_d[:, :])
            # xt on the HWDGE queues (sync + scalar): the gpsimd SWDGE queue
            # is slow and stalls while the gpsimd engine computes (the
            # iota), which starved the projection when xt shared it
            for half in range(2):
                for j in range(4):
                    nc.sync.dma_start(
                        out=xt_sb[j][:, half * 512:(half + 1) * 512],
                        in_=xt_d[j * 128:(j + 1) * 128, half * 512:(half + 1) * 512],
                    )
            # xtq blocks 0+1 (slots 0/1): j2/j3 ride gpsimd ahead of the iota
            for j, eng in [(0, nc.sync), (1, nc.sync), (2, nc.gpsimd), (3, nc.gpsimd)]:
                eng.dma_start(
                    out=xtq_sb[j][:, 0:1024],
                    in_=xtq_d[j * 128:(j + 1) * 128, 0:1024],
                )
            # iota ramp v[p,t,j] = -128t + j - p (f32 exact for small ints)
            nc.gpsimd.iota(
                iota_sb[:], pattern=[[-128, 8], [1, 512]], base=0,
                channel_multiplier=-1, allow_small_or_imprecise_dtypes=True,
            )
            def emit_xt_rest(c0, c1):
                for j in range(4):
                    nc.sync.dma_start(
                        out=xt_sb[j][:, c0:c1],
                        in_=xt_d[j * 128:(j + 1) * 128, c0:c1],
                    )

            def emit_xtq_rest():
                for j in range(4):
                    nc.gpsimd.dma_start(
                        out=xtq_sb[j][:, 1024:2048],
                        in_=xtq_d[j * 128:(j + 1) * 128, 1024:2048],
                    )

            # PE p-state warmup: the tensor engine needs sustained activity
            # before it clocks up, and real work only arrives once the first
            # x tiles land (~14us). Burn the DMA lead-in on dependency-free
            # dummy matmuls so the ramp happens before the projection starts.
            # The warm memset is the FIRST vector instruction so the dummies
            # start as early as possible.
            warm_sb = small.tile([128, 512], BF16, tag="warm")
            nc.vector.memset(warm_sb[:], 0.0)

            def emit_warm(n):
                for _ in range(n):
                    wp = projps.tile([128, 512], F32, tag="proj")
                    nc.tensor.matmul(
                        wp[:], lhsT=warm_sb[:, 0:128], rhs=warm_sb[:],
                        start=True, stop=True,
                    )

            emit_warm(16)
            nc.vector.memset(vaug3[:, :, 64:65], 1.0)
            nc.vector.memset(ones_sb[:], 1.0)

            def emit_mask(s):
                # keep-mask[p,t,j] = 1.0 where (-128t + j - p) >= thr[s] else 0
                nc.vector.tensor_scalar(
                    out=mask_sb[:, s, :],
                    in0=iota_sb[:].rearrange("p a b -> p (a b)"),
                    scalar1=fpack_sb[:, 2 + s:3 + s],
                    scalar2=None,
                    op0=mybir.AluOpType.is_ge,
                )

            def emit_kv_chunk(c):
                # seq cols [c*1024,(c+1)*1024) = k-tiles 8c..8c+7
                for half in range(2):
                    col = c * 1024 + half * 512
                    kvp = projps.tile([128, 512], F32, tag="proj")
                    for j in range(4):
                        nc.tensor.matmul(
                            kvp[:],
                            lhsT=wpack_sb[:, j * 128:(j + 1) * 128],
                            rhs=xt_sb[j][:, col:col + 512],
                            start=(j == 0),
                            stop=(j == 3),
                        )
                    nc.vector.tensor_scalar_add(
                        kvt_sb[:, col:col + 512], kvp[:], bkv_ap
                    )
            def emit_vtr(c, eng):
                # V^T -> V natural via the XBAR transpose DMA, one per
                # 512-col half so the slot's first AV group isn't gated on
                # the whole chunk. The issuing engine stalls on the DMA's
                # input semaphore (kvt's DVE write), so this must be emitted
                # only after every DMA (on the same engine AND queue) whose
                # data is needed sooner.
                for half in range(2):
                    col = c * 1024 + half * 512
                    kt0 = 8 * c + 4 * half
                    eng.dma_start_transpose(
                        out=vaug3[:, kt0:kt0 + 4, 0:64],
                        in_=kvt_sb[0:64, col:col + 512],
                    )

            def emit_q_chunk(c):
                # Q^T lands on partitions 64:128 so the QK matmul can take
                # K^T straight from kvt's lower half (matching base partition)
                for blk in (2 * c, 2 * c + 1):
                    qp = projps.tile([128, 512], F32, tag="proj")
                    for j in range(4):
                        nc.tensor.matmul(
                            qp[64:128, :],
                            lhsT=wpack_sb[:, 512 + j * H:512 + (j + 1) * H],
                            rhs=xtq_sb[j][:, blk * 512:(blk + 1) * 512],
                            start=(j == 0),
                            stop=(j == 3),
                        )
                    nc.vector.tensor_scalar_add(
                        qtp_sb[64:128, blk * 512:(blk + 1) * 512],
                        qp[64:128, :], bq_ap
                    )

            def emit_slot(s):
                nkt = NKT[s]
                ngrp = nkt // 2
                otp = otps.tile([H + 1, 512], F32, tag="otp")
                pts = {}
                for g in range(ngrp + 1):
                    if g < ngrp:
                        stp = stps.tile([128, 1024], F32, tag="stp")
                        for u in range(2):
                            kt = 2 * g + u
                            nc.tensor.matmul(
                                stp[:, u * 512:(u + 1) * 512],
                                lhsT=kvt_sb[64:128, kt * 128:(kt + 1) * 128],
                                rhs=qtp_sb[64:128, s * 512:(s + 1) * 512],
                                start=True,
                                stop=True,
                            )
                        pt = ptp.tile([128, 1024], BF16, tag="pt")
                        nc.scalar.activation(
                            pt[:], stp[:], mybir.ActivationFunctionType.Exp,
                            scale=1.0 / 64.0,
                        )
                        if g >= ngrp - 4:
                            gm = g - (ngrp - 4)
                            nc.vector.tensor_mul(
                                pt[:],
                                pt[:],
                                mask_sb[:, s, gm * 1024:(gm + 1) * 1024],
                            )
                        pts[g] = pt
                    if g >= 1:
                        ptm = pts.pop(g - 1)
                        for u in range(2):
                            kt = 2 * (g - 1) + u
                            nc.tensor.matmul(
                                otp[:],
                                lhsT=vaug3[:, kt, 0:65],
                                rhs=ptm[:, u * 512:(u + 1) * 512],
                                start=(kt == 0),
                                stop=(kt == nkt - 1),
                            )
                # epilogue A: stash ot (incl. denominator row 64) in bf16;
                # dnrow is a partition-0 copy of the denominator row for the
                # K=1 transpose matmuls
                ot_sb = epi.tile([H + 1, 512], BF16, tag="ot_sb")
                dnrow = epi.tile([1, 512], BF16, tag="dnrow")
                # for the final slot these copies are the kernel tail; the
                # scalar engine has finished its exps by then and is idle
                eng = nc.scalar if s == 3 else nc.vector
                if s == 3:
                    eng.copy(ot_sb[:], otp[:])
                    eng.copy(dnrow[:], otp[H:H + 1, :])
                else:
                    eng.tensor_copy(ot_sb[:], otp[:])
                    eng.tensor_copy(dnrow[:], otp[H:H + 1, :])
                return ot_sb, dnrow

            deferred_outs = []

            def emit_epi_b(s, ot_sb, dnrow):
                # the final slot's epilogue is the kernel tail: fan its scale
                # and output DMA across two engines/queues each
                last = s == 3
                for t in range(4):
                    dnp = projps.tile([128, 1], F32, tag="proj")
                    nc.tensor.matmul(
                        dnp[:],
                        lhsT=dnrow[:, t * 128:(t + 1) * 128],
                        rhs=ones_sb[:],
                        start=True,
                        stop=True,
                    )
                    recip = epi.tile([128, 1], F32, tag="recip")
                    nc.vector.reciprocal(recip[:], dnp[:])
                    yp = projps.tile([128, 512], F32, tag="proj")
                    nc.tensor.matmul(
                        yp[:],
                        lhsT=ot_sb[:, t * 128:(t + 1) * 128],
                        rhs=wo_sb[:],
                        start=True,
                        stop=True,
                    )
                    ysb = epi.tile([128, 512], BF16, tag="ysb", bufs=14)
                    if last and t % 2 == 1:
                        nc.scalar.activation(
                            ysb[:], yp[:], mybir.ActivationFunctionType.Copy,
                            scale=recip[:],
                        )
                    else:
                        nc.vector.tensor_scalar_mul(ysb[:], yp[:], recip[:])
                    if last:
                        out_eng = nc.scalar if t % 2 == 1 else nc.sync
                        out_eng.dma_start(
                            out=out_d[s * 512 + t * 128:s * 512 + (t + 1) * 128, :],
                            in_=ysb[:],
                        )
                    else:
                        # defer the out DMA: issuing it now would block the
                        # sync engine on the ysb semaphore while load DMAs
                        # still need issuing
                        deferred_outs.append((s, t, ysb))

            emit_kv_chunk(0)
            emit_xt_rest(1024, 2048)
            emit_xt_rest(2048, 4096)
            # slot 0's V transpose on scalar: its engine-block window ends
            # exactly when slot 0's first exp could start anyway
            emit_vtr(0, nc.scalar)
            emit_warm(3)
            emit_q_chunk(0)
            emit_warm(2)
            emit_mask(0)
            emit_mask(1)
            ot0 = emit_slot(0)
            emit_kv_chunk(1)
            emit_xtq_rest()
            emit_vtr(1, nc.sync)
            emit_mask(2)
            emit_mask(3)
            emit_epi_b(0, *ot0)
            ot1 = emit_slot(1)
            emit_kv_chunk(2)
            emit_q_chunk(1)
            emit_vtr(2, nc.sync)
            ot2 = emit_slot(2)
            emit_kv_chunk(3)
            emit_vtr(3, nc.sync)
            # epilogues 1+2 run here: slot 3's first 12 groups need no
            # vector work, so the bunched recip/scale ops cannot stall it
            emit_epi_b(1, *ot1)
            emit_epi_b(2, *ot2)
            for s, t, ysb in deferred_outs:
                nc.sync.dma_start(
                    out=out_d[s * 512 + t * 128:s * 512 + (t + 1) * 128, :],
                    in_=ysb[:],
                )
            deferred_outs.clear()
            ot3 = emit_slot(3)
            emit_epi_b(3, *ot3)

    nc.compile()
    return nc


_NC_CACHE = {}


def _make_in_maps(x, Wq, bq, Wk, bk, Wv, bv, Wo, bo):
    wkv = np.concatenate([Wv, Wk], axis=1)                    # (512, 128)
    wo_aug = np.concatenate([Wo, (bv @ Wo + bo)[None, :]], axis=0).astype(ml_dtypes.bfloat16)
    # wpack[p, j*128+m] = wkv[j*128+p, m]; wpack[p, 512+j*64+h] = Wq[j*128+p, h]
    wpack = np.zeros((128, 768), np.float32)
    for j in range(4):
        wpack[:, j * 128:(j + 1) * 128] = wkv[j * 128:(j + 1) * 128, :]
        wpack[:, 512 + j * H:512 + (j + 1) * H] = Wq[j * 128:(j + 1) * 128, :]
    wpack = wpack.astype(ml_dtypes.bfloat16)

    in_maps = []
    for c in range(8):
        b = c // 2
        blocks = BLOCKS_EVEN if c % 2 == 0 else BLOCKS_ODD
        xt = np.ascontiguousarray(x[b].T).astype(ml_dtypes.bfloat16)  # (512, 4096)
        qcols = np.concatenate(
            [np.arange(blk * QB, (blk + 1) * QB) for blk in blocks]
        )
        xtq = np.ascontiguousarray(xt[:, qcols])               # (512, 2048)
        fpack = np.zeros((128, 6), np.float32)
        fpack[64:, 0] = bk
        fpack[0:64, 1] = bq
        for s in range(4):
            fpack[:, 2 + s] = (NKT[s] - 8) * 128 - 512 * blocks[s]
        in_maps.append({
            "xt": xt,
            "xtq": xtq,
            "wpack": wpack,
            "wo": wo_aug,
            "fpack": fpack,
        })
    return in_maps


def kernel(x, Wq, bq, Wk, bk, Wv, bv, Wo, bo):
    global LAST_EXEC_TIME_NS, LAST_RESULTS
    x = np.asarray(x, dtype=np.float32)
    Wq, bq = np.asarray(Wq, np.float32), np.asarray(bq, np.float32)
    Wk, bk = np.asarray(Wk, np.float32), np.asarray(bk, np.float32)
    Wv, bv = np.asarray(Wv, np.float32), np.asarray(bv, np.float32)
    Wo, bo = np.asarray(Wo, np.float32), np.asarray(bo, np.float32)

    if "nc" not in _NC_CACHE:
        _NC_CACHE["nc"] = _build_nc()
    nc = _NC_CACHE["nc"]

    in_maps = _make_in_maps(x, Wq, bq, Wk, bk, Wv, bv, Wo, bo)

    trace = os.environ.get("KERNEL_TRACE", "1") == "1"
    if trace:
        trace = _install_ntff_hook()
    tmpdir = os.environ.get("KERNEL_TRACE_DIR") or None
    try:
        res = run_bass_kernel_spmd(
            nc, in_maps, core_ids=list(range(8)), trace=trace, tmpdir=tmpdir
        )
    except Exception:
        if not trace:
            raise
        res = run_bass_kernel_spmd(nc, in_maps, core_ids=list(range(8)), trace=False)
    LAST_EXEC_TIME_NS = res.exec_time_ns
    LAST_RESULTS = res

    out = np.empty((B, S, D), np.float32)
    for c in range(8):
        b = c // 2
        blocks = BLOCKS_EVEN if c % 2 == 0 else BLOCKS_ODD
        shard = res.results[c]["out"].astype(np.float32)
        for sidx, blk in enumerate(blocks):
            out[b, blk * QB:(blk + 1) * QB, :] = shard[sidx * QB:(sidx + 1) * QB, :]
    return out


# revision 68
# speedup vs baseline: 1.0015x; 1.0015x over previous
"""Causal single-head attention layer on 8 TRN2 NeuronCores.

Reference (per batch b):
  Q = x@Wq+bq; K = x@Wk+bk; V = x@Wv+bv        (S=4096, D=512, H=64)
  S = Q K^T / sqrt(S);  P = softmax(S + causal_mask);  out = (P V) @ Wo + bo

Sharding: 8 cores = 4 batches x 2 halves. Each core owns 4 query-blocks
of 512 rows of its batch in ASCENDING causal order: even cores take
blocks [0,3,4,7], odd take [1,2,5,6]. SPMD structural k-tile counts per
slot NKT=[8,16,24,32] cover both parities; over-structural k-tiles and
the causal boundary are killed by an additive -1e5 mask generated
ON-CHIP from an iota ramp compared against a per-core threshold input
(thr[s] = (NKT[s]-8)*128 - 512*blk_s) -- no mask DMA.

Schedule (per-engine instruction streams execute in emission order):
KV-projection chunks, Q-projection, attention slots and epilogues are
interleaved so slot s runs as soon as k-tiles 0..NKT[s]-1 exist;
ascending slot order makes slot 0 ready after 1/4 of the projection.
Hard-won scheduling rules baked in here:
 - a dma_start BLOCKS its issuing engine until the DMA's input data is
   ready, and each DMA queue is FIFO -- so every load DMA is emitted
   before any DMA that depends on on-chip compute (V^T XBAR
   transposes, output tiles), and output DMAs for slots 0-2 are
   deferred to the pre-slot-3 bunch;
 - the gpsimd SWDGE queue stalls while the gpsimd engine computes, so
   all xt bulk goes over the sync HWDGE queue (weights over scalar's);
 - the PE downclocks when idle and takes ~2.5us of heavy activity to
   reach full speed, so dependency-free warm-up matmuls burn the
   ~13us DMA lead-in and keep the ramp alive through early data gaps;
 - epilogue recip/scale work is bunched right before slot 3, whose
   first 12 groups need no vector work, so it can never stall an AV.
Engine split: scalar = exp ACTs (+slot-3 tail); sync = xt/out DMA;
gpsimd = xtq DMA + iota; vector = bias/scale/mask work.

Per group of 2 k-tiles: S^T [128k,1024] = K^T.T @ Q^T (bf16 PE, K^T
read straight from kvt[64:128] with Q^T placed at base partition 64)
-> exp via ACT (scale=1/64 folded) -> P^T bf16 -> multiplicative
{0,1} mask (tail groups only, DVE) -> AV accumulate otp[65,512]
(V_aug carries a ones column so the softmax denominator falls out of
row 64; V natural layout produced by XBAR transpose DMAs). QK/AV are
software-pipelined (QK(g+1) before AV(g)) so the PE does not wait on
the exp. Epilogue: denominator row -> partitions via K=1 matmul,
reciprocal, y = ot^T @ [Wo; bv@Wo+bo] scaled by 1/denom -> bf16 out
DMA (host casts to f32). Softmax max-subtraction skipped: |S/64| <~ 1
so exp is safe.
"""

import os

os.environ.setdefault("MYCRO_LOCAL_CACHE", "1")

import numpy as np
import ml_dtypes

import concourse.mybir as mybir
import concourse.tile as tile
from concourse import bacc
from concourse.bass_utils import run_bass_kernel_spmd

F32 = mybir.dt.float32
BF16 = mybir.dt.bfloat16

B, S, D, H = 4, 4096, 512, 64
QB = 512
NKT = [8, 16, 24, 32]          # structural k-tiles per slot (ascending)
BLOCKS_EVEN = [0, 3, 4, 7]
BLOCKS_ODD = [1, 2, 5, 6]

LAST_EXEC_TIME_NS = None
LAST_RESULTS = None


def _install_ntff_hook():
    import sys
    import types
    try:
        from antenv.axon_hooks import get_axon_ntff_profile_hook  # noqa: F401
        return True
    except ImportError:
        pass
    try:
        import trn_agent_boot.trn_boot as _tb
        hook = _tb._ntff_profile_via_ctypes("/opt/axon/libaxon_pjrt.so")
        if hook is None:
            return False
        mod = types.ModuleType("antenv.axon_hooks")
        mod.get_axon_ntff_profile_hook = lambda: hook
        mod.set_axon_ntff_profile_hook = lambda h: None
        sys.modules["antenv.axon_hooks"] = mod
        return True
    except Exception:
        return False


def _build_nc():
    nc = bacc.Bacc(
        "TRN2",
        target_bir_lowering=False,
        debug=False,
        enable_asserts=False,
        num_devices=8,
    )

    xt_d = nc.dram_tensor("xt", [D, S], BF16, kind="ExternalInput")
    xtq_d = nc.dram_tensor("xtq", [D, 4 * QB], BF16, kind="ExternalInput")
    wpack_d = nc.dram_tensor("wpack", [128, 768], BF16, kind="ExternalInput")
    wo_d = nc.dram_tensor("wo", [H + 1, D], BF16, kind="ExternalInput")
    fpack_d = nc.dram_tensor("fpack", [128, 6], F32, kind="ExternalInput")
    out_d = nc.dram_tensor("out", [4 * QB, D], BF16, kind="ExternalOutput")

    with tile.TileContext(nc) as tc:
        with (
            tc.tile_pool(name="big", bufs=1) as big,
            tc.tile_pool(name="small", bufs=1) as small,
            tc.tile_pool(name="projps", bufs=2, space="PSUM") as projps,
            tc.tile_pool(name="stps", bufs=2, space="PSUM") as stps,
            tc.tile_pool(name="otps", bufs=2, space="PSUM") as otps,
            tc.tile_pool(name="ptp", bufs=4) as ptp,
            tc.tile_pool(name="epi", bufs=2) as epi,
        ):
            # ---- persistent SBUF ----
            xt_sb = [big.tile([128, S], BF16, name=f"xt{j}", tag=f"xt{j}") for j in range(4)]
            xtq_sb = [big.tile([128, 4 * QB], BF16, name=f"xtq{j}", tag=f"xtq{j}") for j in range(4)]
            kvt_sb = big.tile([128, S], BF16, tag="kvt")      # rows 0:64 V^T, 64:128 K^T
            qtp_sb = big.tile([128, 4 * QB], BF16, tag="qtp")  # Q^T on partitions 64:128
            vaug_sb = big.tile([128, 32 * 80], BF16, tag="vaug")
            iota_sb = big.tile([128, 8, 512], F32, tag="iota")
            mask_sb = big.tile([128, 4, 4096], BF16, tag="mask")
            wpack_sb = small.tile([128, 768], BF16, tag="wpack")
            wo_sb = small.tile([H + 1, D], BF16, tag="wo")
            fpack_sb = small.tile([128, 6], F32, tag="fpack")
            ones_sb = small.tile([1, 1], BF16, tag="ones")

            vaug3 = vaug_sb[:].rearrange("p (k c) -> p k c", c=80)
            bkv_ap = fpack_sb[:, 0:1]
            bq_ap = fpack_sb[0:64, 1:2]

            # ---- input DMAs ----
            # scalar: packed weights only (its queue is idle until the
            # first exp ACT; keeps sync/gpsimd free for bulk x)
            nc.scalar.dma_start(out=wpack_sb[:], in_=wpack_d[:, :])
            nc.scalar.dma_start(out=fpack_sb[:], in_=fpack_d[:, :])
            nc.scalar.dma_start(out=wo_sb[:], in_=wo_d[:, :])
            # xt on the HWDGE queues (sync + scalar): the gpsimd SWDGE queue
            # is slow and stalls while the gpsimd engine computes (the
            # iota), which starved the projection when xt shared it
            for half in range(2):
                for j in range(4):
                    nc.sync.dma_start(
                        out=xt_sb[j][:, half * 512:(half + 1) * 512],
                        in_=xt_d[j * 128:(j + 1) * 128, half * 512:(half + 1) * 512],
                    )
            # xtq blocks 0+1 (slots 0/1): j2/j3 ride gpsimd ahead of the iota
            for j, eng in [(0, nc.sync), (1, nc.sync), (2, nc.gpsimd), (3, nc.gpsimd)]:
                eng.dma_start(
                    out=xtq_sb[j][:, 0:1024],
                    in_=xtq_d[j * 128:(j + 1) * 128, 0:1024],
                )
            # iota ramp v[p,t,j] = -128t + j - p (f32 exact for small ints)
            nc.gpsimd.iota(
                iota_sb[:], pattern=[[-128, 8], [1, 512]], base=0,
                channel_multiplier=-1, allow_small_or_imprecise_dtypes=True,
            )
            def emit_xt_rest(c0, c1, engs=None):
                engs = engs or [nc.sync] * 4
                for j in range(4):
                    engs[j].dma_start(
                        out=xt_sb[j][:, c0:c1],
                        in_=xt_d[j * 128:(j + 1) * 128, c0:c1],
                    )

            def emit_xtq_rest():
                for j in range(4):
                    nc.gpsimd.dma_start(
                        out=xtq_sb[j][:, 1024:2048],
                        in_=xtq_d[j * 128:(j + 1) * 128, 1024:2048],
                    )

            # PE p-state warmup: the tensor engine needs sustained activity
            # before it clocks up, and real work only arrives once the first
            # x tiles land (~14us). Burn the DMA lead-in on dependency-free
            # dummy matmuls so the ramp happens before the projection starts.
            # The warm memset is the FIRST vector instruction so the dummies
            # start as early as possible.
            warm_sb = small.tile([128, 512], BF16, tag="warm")
            nc.vector.memset(warm_sb[:], 0.0)

            def emit_warm(n):
                for _ in range(n):
                    wp = projps.tile([128, 512], F32, tag="proj")
                    nc.tensor.matmul(
                        wp[:], lhsT=warm_sb[:, 0:128], rhs=warm_sb[:],
                        start=True, stop=True,
                    )

            emit_warm(16)
            nc.vector.memset(vaug3[:, :, 64:65], 1.0)
            nc.vector.memset(ones_sb[:], 1.0)

            def emit_mask(s):
                # keep-mask[p,t,j] = 1.0 where (-128t + j - p) >= thr[s] else 0
                nc.vector.tensor_scalar(
                    out=mask_sb[:, s, :],
                    in0=iota_sb[:].rearrange("p a b -> p (a b)"),
                    scalar1=fpack_sb[:, 2 + s:3 + s],
                    scalar2=None,
                    op0=mybir.AluOpType.is_ge,
                )

            def emit_kv_chunk(c):
                # seq cols [c*1024,(c+1)*1024) = k-tiles 8c..8c+7
                for half in range(2):
                    col = c * 1024 + half * 512
                    kvp = projps.tile([128, 512], F32, tag="proj")
                    for j in range(4):
                        nc.tensor.matmul(
                            kvp[:],
                            lhsT=wpack_sb[:, j * 128:(j + 1) * 128],
                            rhs=xt_sb[j][:, col:col + 512],
                            start=(j == 0),
                            stop=(j == 3),
                        )
                    nc.vector.tensor_scalar_add(
                        kvt_sb[:, col:col + 512], kvp[:], bkv_ap
                    )
            def emit_vtr(c, eng, pieces=2):
                # V^T -> V natural via the XBAR transpose DMA, in pieces so
                # the slot's first AV group isn't gated on the whole chunk.
                # The issuing engine stalls on the DMA's input semaphore
                # (kvt's DVE write), so this must be emitted only after
                # every DMA (on the same engine AND queue) whose data is
                # needed sooner.
                w = 1024 // pieces
                for p in range(pieces):
                    col = c * 1024 + p * w
                    kt0 = 8 * c + p * (w // 128)
                    eng.dma_start_transpose(
                        out=vaug3[:, kt0:kt0 + w // 128, 0:64],
                        in_=kvt_sb[0:64, col:col + w],
                    )

            def emit_q_chunk(c):
                # Q^T lands on partitions 64:128 so the QK matmul can take
                # K^T straight from kvt's lower half (matching base partition)
                for blk in (2 * c, 2 * c + 1):
                    qp = projps.tile([128, 512], F32, tag="proj")
                    for j in range(4):
                        nc.tensor.matmul(
                            qp[64:128, :],
                            lhsT=wpack_sb[:, 512 + j * H:512 + (j + 1) * H],
                            rhs=xtq_sb[j][:, blk * 512:(blk + 1) * 512],
                            start=(j == 0),
                            stop=(j == 3),
                        )
                    nc.vector.tensor_scalar_add(
                        qtp_sb[64:128, blk * 512:(blk + 1) * 512],
                        qp[64:128, :], bq_ap
                    )

            def emit_slot(s):
                nkt = NKT[s]
                ngrp = nkt // 2
                otp = otps.tile([H + 1, 512], F32, tag="otp")
                pts = {}
                for g in range(ngrp + 1):
                    if g < ngrp:
                        stp = stps.tile([128, 1024], F32, tag="stp")
                        for u in range(2):
                            kt = 2 * g + u
                            nc.tensor.matmul(
                                stp[:, u * 512:(u + 1) * 512],
                                lhsT=kvt_sb[64:128, kt * 128:(kt + 1) * 128],
                                rhs=qtp_sb[64:128, s * 512:(s + 1) * 512],
                                start=True,
                                stop=True,
                            )
                        pt = ptp.tile([128, 1024], BF16, tag="pt")
                        nc.scalar.activation(
                            pt[:], stp[:], mybir.ActivationFunctionType.Exp,
                            scale=1.0 / 64.0,
                        )
                        if g >= ngrp - 4:
                            gm = g - (ngrp - 4)
                            nc.vector.tensor_mul(
                                pt[:],
                                pt[:],
                                mask_sb[:, s, gm * 1024:(gm + 1) * 1024],
                            )
                        pts[g] = pt
                    if g >= 1:
                        ptm = pts.pop(g - 1)
                        for u in range(2):
                            kt = 2 * (g - 1) + u
                            nc.tensor.matmul(
                                otp[:],
                                lhsT=vaug3[:, kt, 0:65],
                                rhs=ptm[:, u * 512:(u + 1) * 512],
                                start=(kt == 0),
                                stop=(kt == nkt - 1),
                            )
                # epilogue A: stash ot (incl. denominator row 64) in bf16;
                # dnrow is a partition-0 copy of the denominator row for the
                # K=1 transpose matmuls
                ot_sb = epi.tile([H + 1, 512], BF16, tag="ot_sb")
                dnrow = epi.tile([1, 512], BF16, tag="dnrow")
                # for the final slot these copies are the kernel tail; the
                # scalar engine has finished its exps by then and is idle
                if s == 3:
                    nc.vector.tensor_copy(ot_sb[:], otp[:])
                    nc.scalar.copy(dnrow[:], otp[H:H + 1, :])
                else:
                    nc.vector.tensor_copy(ot_sb[:], otp[:])
                    nc.vector.tensor_copy(dnrow[:], otp[H:H + 1, :])
                return ot_sb, dnrow

            deferred_outs = []

            def emit_epi_b(s, ot_sb, dnrow):
                # the final slot's epilogue is the kernel tail: fan its scale
                # and output DMA across two engines/queues each
                last = s == 3
                for t in range(4):
                    dnp = projps.tile([128, 1], F32, tag="proj")
                    nc.tensor.matmul(
                        dnp[:],
                        lhsT=dnrow[:, t * 128:(t + 1) * 128],
                        rhs=ones_sb[:],
                        start=True,
                        stop=True,
                    )
                    recip = epi.tile([128, 1], F32, tag="recip")
                    nc.vector.reciprocal(recip[:], dnp[:])
                    yp = projps.tile([128, 512], F32, tag="proj")
                    nc.tensor.matmul(
                        yp[:],
                        lhsT=ot_sb[:, t * 128:(t + 1) * 128],
                        rhs=wo_sb[:],
                        start=True,
                        stop=True,
                    )
                    ysb = epi.tile([128, 512], BF16, tag="ysb", bufs=14)
                    if last and t % 2 == 1:
                        nc.scalar.activation(
                            ysb[:], yp[:], mybir.ActivationFunctionType.Copy,
                            scale=recip[:],
                        )
                    else:
                        nc.vector.tensor_scalar_mul(ysb[:], yp[:], recip[:])
                    if last:
                        out_eng = nc.scalar if t % 2 == 1 else nc.sync
                        out_eng.dma_start(
                            out=out_d[s * 512 + t * 128:s * 512 + (t + 1) * 128, :],
                            in_=ysb[:],
                        )
                    else:
                        # defer the out DMA: issuing it now would block the
                        # sync engine on the ysb semaphore while load DMAs
                        # still need issuing
                        deferred_outs.append((s, t, ysb))

            emit_kv_chunk(0)
            emit_xt_rest(1024, 2048)
            emit_xt_rest(2048, 4096)
            # slot 0's V transpose on scalar: its engine-block window ends
            # exactly when slot 0's first exp could start anyway
            emit_vtr(0, nc.scalar)
            emit_warm(3)
            emit_q_chunk(0)
            emit_warm(2)
            emit_mask(0)
            emit_mask(1)
            ot0 = emit_slot(0)
            emit_kv_chunk(1)
            emit_xtq_rest()
            emit_vtr(1, nc.sync)
            emit_mask(2)
            emit_mask(3)
            emit_epi_b(0, *ot0)
            ot1 = emit_slot(1)
            emit_kv_chunk(2)
            emit_q_chunk(1)
            emit_vtr(2, nc.sync)
            ot2 = emit_slot(2)
            emit_kv_chunk(3)
            emit_vtr(3, nc.sync)
            # epilogues 1+2 run here: slot 3's first 12 groups need no
            # vector work, so the bunched recip/scale ops cannot stall it
            emit_epi_b(1, *ot1)
            emit_epi_b(2, *ot2)
            for s, t, ysb in deferred_outs:
                nc.sync.dma_start(
                    out=out_d[s * 512 + t * 128:s * 512 + (t + 1) * 128, :],
                    in_=ysb[:],
                )
            deferred_outs.clear()
            ot3 = emit_slot(3)
            emit_epi_b(3, *ot3)

    nc.compile()
    return nc


_NC_CACHE = {}


def _make_in_maps(x, Wq, bq, Wk, bk, Wv, bv, Wo, bo):
    wkv = np.concatenate([Wv, Wk], axis=1)                    # (512, 128)
    wo_aug = np.concatenate([Wo, (bv @ Wo + bo)[None, :]], axis=0).astype(ml_dtypes.bfloat16)
    # wpack[p, j*128+m] = wkv[j*128+p, m]; wpack[p, 512+j*64+h] = Wq[j*128+p, h]
    wpack = np.zeros((128, 768), np.float32)
    for j in range(4):
        wpack[:, j * 128:(j + 1) * 128] = wkv[j * 128:(j + 1) * 128, :]
        wpack[:, 512 + j * H:512 + (j + 1) * H] = Wq[j * 128:(j + 1) * 128, :]
    wpack = wpack.astype(ml_dtypes.bfloat16)

    in_maps = []
    for c in range(8):
        b = c // 2
        blocks = BLOCKS_EVEN if c % 2 == 0 else BLOCKS_ODD
        xt = np.ascontiguousarray(x[b].T).astype(ml_dtypes.bfloat16)  # (512, 4096)
        qcols = np.concatenate(
            [np.arange(blk * QB, (blk + 1) * QB) for blk in blocks]
        )
        xtq = np.ascontiguousarray(xt[:, qcols])               # (512, 2048)
        fpack = np.zeros((128, 6), np.float32)
        fpack[64:, 0] = bk
        fpack[0:64, 1] = bq
        for s in range(4):
            fpack[:, 2 + s] = (NKT[s] - 8) * 128 - 512 * blocks[s]
        in_maps.append({
            "xt": xt,
            "xtq": xtq,
            "wpack": wpack,
            "wo": wo_aug,
            "fpack": fpack,
        })
    return in_maps


def kernel(x, Wq, bq, Wk, bk, Wv, bv, Wo, bo):
    global LAST_EXEC_TIME_NS, LAST_RESULTS
    x = np.asarray(x, dtype=np.float32)
    Wq, bq = np.asarray(Wq, np.float32), np.asarray(bq, np.float32)
    Wk, bk = np.asarray(Wk, np.float32), np.asarray(bk, np.float32)
    Wv, bv = np.asarray(Wv, np.float32), np.asarray(bv, np.float32)
    Wo, bo = np.asarray(Wo, np.float32), np.asarray(bo, np.float32)

    if "nc" not in _NC_CACHE:
        _NC_CACHE["nc"] = _build_nc()
    nc = _NC_CACHE["nc"]

    in_maps = _make_in_maps(x, Wq, bq, Wk, bk, Wv, bv, Wo, bo)

    trace = os.environ.get("KERNEL_TRACE", "1") == "1"
    if trace:
        trace = _install_ntff_hook()
    tmpdir = os.environ.get("KERNEL_TRACE_DIR") or None
    try:
        res = run_bass_kernel_spmd(
            nc, in_maps, core_ids=list(range(8)), trace=trace, tmpdir=tmpdir
        )
    except Exception:
        if not trace:
            raise
        res = run_bass_kernel_spmd(nc, in_maps, core_ids=list(range(8)), trace=False)
    LAST_EXEC_TIME_NS = res.exec_time_ns
    LAST_RESULTS = res

    out = np.empty((B, S, D), np.float32)
    for c in range(8):
        b = c // 2
        blocks = BLOCKS_EVEN if c % 2 == 0 else BLOCKS_ODD
        shard = res.results[c]["out"].astype(np.float32)
        for sidx, blk in enumerate(blocks):
            out[b, blk * QB:(blk + 1) * QB, :] = shard[sidx * QB:(sidx + 1) * QB, :]
    return out


# revision 70
# speedup vs baseline: 1.0490x; 1.0475x over previous
"""Causal single-head attention layer on 8 TRN2 NeuronCores.

Reference (per batch b):
  Q = x@Wq+bq; K = x@Wk+bk; V = x@Wv+bv        (S=4096, D=512, H=64)
  S = Q K^T / sqrt(S);  P = softmax(S + causal_mask);  out = (P V) @ Wo + bo

Sharding: 8 cores = 4 batches x 2 halves. Each core owns 4 query-blocks
of 512 rows of its batch in ASCENDING causal order: even cores take
blocks [0,3,4,7], odd take [1,2,5,6]. SPMD structural k-tile counts per
slot NKT=[8,16,24,32] cover both parities; over-structural k-tiles and
the causal boundary are killed by an additive -1e5 mask generated
ON-CHIP from an iota ramp compared against a per-core threshold input
(thr[s] = (NKT[s]-8)*128 - 512*blk_s) -- no mask DMA.

Schedule (per-engine instruction streams execute in emission order):
KV-projection chunks, Q-projection, attention slots and epilogues are
interleaved so slot s runs as soon as k-tiles 0..NKT[s]-1 exist;
ascending slot order makes slot 0 ready after 1/4 of the projection.
Hard-won scheduling rules baked in here:
 - a dma_start BLOCKS its issuing engine until the DMA's input data is
   ready, and each DMA queue is FIFO -- so every load DMA is emitted
   before any DMA that depends on on-chip compute (V^T XBAR
   transposes, output tiles), and output DMAs for slots 0-2 are
   deferred to the pre-slot-3 bunch;
 - the gpsimd SWDGE queue stalls while the gpsimd engine computes, so
   all xt bulk goes over the sync HWDGE queue (weights over scalar's);
 - the PE downclocks when idle and takes ~2.5us of heavy activity to
   reach full speed, so dependency-free warm-up matmuls burn the
   ~13us DMA lead-in and keep the ramp alive through early data gaps;
 - epilogue recip/scale work is bunched right before slot 3, whose
   first 12 groups need no vector work, so it can never stall an AV.
Engine split: scalar = exp ACTs (+slot-3 tail); sync = xt/out DMA;
gpsimd = xtq DMA + iota; vector = bias/scale/mask work.

Per group of 2 k-tiles: S^T [128k,1024] = K^T.T @ Q^T (bf16 PE, K^T
read straight from kvt[64:128] with Q^T placed at base partition 64)
-> exp via ACT (scale=1/64 folded) -> P^T bf16 -> multiplicative
{0,1} mask (tail groups only, DVE) -> AV accumulate otp[65,512]
(V_aug carries a ones column so the softmax denominator falls out of
row 64; V natural layout produced by XBAR transpose DMAs). QK/AV are
software-pipelined (QK(g+1) before AV(g)) so the PE does not wait on
the exp. Epilogue: denominator row -> partitions via K=1 matmul,
reciprocal, y = ot^T @ [Wo; bv@Wo+bo] scaled by 1/denom -> bf16 out
DMA (host casts to f32). Softmax max-subtraction skipped: |S/64| <~ 1
so exp is safe.
"""

import os

os.environ.setdefault("MYCRO_LOCAL_CACHE", "1")

import numpy as np
import ml_dtypes

import concourse.mybir as mybir
import concourse.tile as tile
from concourse import bacc
from concourse.bass_utils import run_bass_kernel_spmd

F32 = mybir.dt.float32
BF16 = mybir.dt.bfloat16

B, S, D, H = 4, 4096, 512, 64
QB = 512
NKT = [8, 16, 24, 32]          # structural k-tiles per slot (ascending)
BLOCKS_EVEN = [0, 3, 4, 7]
BLOCKS_ODD = [1, 2, 5, 6]

LAST_EXEC_TIME_NS = None
LAST_RESULTS = None


def _install_ntff_hook():
    import sys
    import types
    try:
        from antenv.axon_hooks import get_axon_ntff_profile_hook  # noqa: F401
        return True
    except ImportError:
        pass
    try:
        import trn_agent_boot.trn_boot as _tb
        hook = _tb._ntff_profile_via_ctypes("/opt/axon/libaxon_pjrt.so")
        if hook is None:
            return False
        mod = types.ModuleType("antenv.axon_hooks")
        mod.get_axon_ntff_profile_hook = lambda: hook
        mod.set_axon_ntff_profile_hook = lambda h: None
        sys.modules["antenv.axon_hooks"] = mod
        return True
    except Exception:
        return False


def _build_nc():
    nc = bacc.Bacc(
        "TRN2",
        target_bir_lowering=False,
        debug=False,
        enable_asserts=False,
        num_devices=8,
    )

    xt_d = nc.dram_tensor("xt", [D, S], BF16, kind="ExternalInput")
    xtq_d = nc.dram_tensor("xtq", [D, 4 * QB], BF16, kind="ExternalInput")
    wpack_d = nc.dram_tensor("wpack", [128, 768], BF16, kind="ExternalInput")
    wo_d = nc.dram_tensor("wo", [H + 1, D], BF16, kind="ExternalInput")
    fpack_d = nc.dram_tensor("fpack", [128, 6], F32, kind="ExternalInput")
    out_d = nc.dram_tensor("out", [4 * QB, D], BF16, kind="ExternalOutput")

    with tile.TileContext(nc) as tc:
        with (
            tc.tile_pool(name="big", bufs=1) as big,
            tc.tile_pool(name="small", bufs=1) as small,
            tc.tile_pool(name="projps", bufs=2, space="PSUM") as projps,
            tc.tile_pool(name="stps", bufs=2, space="PSUM") as stps,
            tc.tile_pool(name="otps", bufs=2, space="PSUM") as otps,
            tc.tile_pool(name="ptp", bufs=4) as ptp,
            tc.tile_pool(name="epi", bufs=2) as epi,
        ):
            # ---- persistent SBUF ----
            xt_sb = [big.tile([128, S], BF16, name=f"xt{j}", tag=f"xt{j}") for j in range(4)]
            xtq_sb = [big.tile([128, 4 * QB], BF16, name=f"xtq{j}", tag=f"xtq{j}") for j in range(4)]
            kvt_sb = big.tile([128, S], BF16, tag="kvt")      # rows 0:64 V^T, 64:128 K^T
            qtp_sb = big.tile([128, 4 * QB], BF16, tag="qtp")  # Q^T on partitions 64:128
            vaug_sb = big.tile([128, 32 * 80], BF16, tag="vaug")
            iota_sb = big.tile([128, 8, 512], F32, tag="iota")
            mask_sb = big.tile([128, 4, 4096], BF16, tag="mask")
            wpack_sb = small.tile([128, 768], BF16, tag="wpack")
            wo_sb = small.tile([H + 1, D], BF16, tag="wo")
            fpack_sb = small.tile([128, 6], F32, tag="fpack")
            ones_sb = small.tile([1, 1], BF16, tag="ones")

            vaug3 = vaug_sb[:].rearrange("p (k c) -> p k c", c=80)
            bkv_ap = fpack_sb[:, 0:1]
            bq_ap = fpack_sb[0:64, 1:2]

            # ---- input DMAs ----
            # scalar: packed weights only (its queue is idle until the
            # first exp ACT; keeps sync/gpsimd free for bulk x)
            nc.scalar.dma_start(out=wpack_sb[:], in_=wpack_d[:, :])
            nc.scalar.dma_start(out=fpack_sb[:], in_=fpack_d[:, :])
            nc.scalar.dma_start(out=wo_sb[:], in_=wo_d[:, :])
            # xt on the HWDGE queues (sync + scalar): the gpsimd SWDGE queue
            # is slow and stalls while the gpsimd engine computes (the
            # iota), which starved the projection when xt shared it
            for half in range(2):
                for j in range(4):
                    nc.sync.dma_start(
                        out=xt_sb[j][:, half * 512:(half + 1) * 512],
                        in_=xt_d[j * 128:(j + 1) * 128, half * 512:(half + 1) * 512],
                    )
            # xtq blocks 0+1 (slots 0/1): j2/j3 ride gpsimd ahead of the iota
            for j, eng in [(0, nc.sync), (1, nc.sync), (2, nc.gpsimd), (3, nc.gpsimd)]:
                eng.dma_start(
                    out=xtq_sb[j][:, 0:1024],
                    in_=xtq_d[j * 128:(j + 1) * 128, 0:1024],
                )
            # iota ramp v[p,t,j] = -128t + j - p (f32 exact for small ints)
            nc.gpsimd.iota(
                iota_sb[:], pattern=[[-128, 8], [1, 512]], base=0,
                channel_multiplier=-1, allow_small_or_imprecise_dtypes=True,
            )
            def emit_xt_rest(c0, c1, engs=None):
                engs = engs or [nc.sync] * 4
                for j in range(4):
                    engs[j].dma_start(
                        out=xt_sb[j][:, c0:c1],
                        in_=xt_d[j * 128:(j + 1) * 128, c0:c1],
                    )

            def emit_xtq_rest():
                for j in range(4):
                    nc.gpsimd.dma_start(
                        out=xtq_sb[j][:, 1024:2048],
                        in_=xtq_d[j * 128:(j + 1) * 128, 1024:2048],
                    )

            # PE p-state warmup: the tensor engine needs sustained activity
            # before it clocks up, and real work only arrives once the first
            # x tiles land (~14us). Burn the DMA lead-in on dependency-free
            # dummy matmuls so the ramp happens before the projection starts.
            # The warm memset is the FIRST vector instruction so the dummies
            # start as early as possible.
            warm_sb = small.tile([128, 512], BF16, tag="warm")
            nc.vector.memset(warm_sb[:], 0.0)

            def emit_warm(n):
                for _ in range(n):
                    wp = projps.tile([128, 512], F32, tag="proj")
                    nc.tensor.matmul(
                        wp[:], lhsT=warm_sb[:, 0:128], rhs=warm_sb[:],
                        start=True, stop=True,
                    )

            emit_warm(16)
            nc.vector.memset(vaug3[:, :, 64:65], 1.0)
            nc.vector.memset(ones_sb[:], 1.0)

            def emit_mask(s):
                # keep-mask[p,t,j] = 1.0 where (-128t + j - p) >= thr[s] else 0
                nc.vector.tensor_scalar(
                    out=mask_sb[:, s, :],
                    in0=iota_sb[:].rearrange("p a b -> p (a b)"),
                    scalar1=fpack_sb[:, 2 + s:3 + s],
                    scalar2=None,
                    op0=mybir.AluOpType.is_ge,
                )

            def emit_kv_chunk(c):
                # seq cols [c*1024,(c+1)*1024) = k-tiles 8c..8c+7
                for half in range(2):
                    col = c * 1024 + half * 512
                    kvp = projps.tile([128, 512], F32, tag="proj")
                    for j in range(4):
                        nc.tensor.matmul(
                            kvp[:],
                            lhsT=wpack_sb[:, j * 128:(j + 1) * 128],
                            rhs=xt_sb[j][:, col:col + 512],
                            start=(j == 0),
                            stop=(j == 3),
                        )
                    nc.vector.tensor_scalar_add(
                        kvt_sb[:, col:col + 512], kvp[:], bkv_ap
                    )
            def emit_vtr(c, eng, pieces=2):
                # V^T -> V natural via the XBAR transpose DMA, in pieces so
                # the slot's first AV group isn't gated on the whole chunk.
                # The issuing engine stalls on the DMA's input semaphore
                # (kvt's DVE write), so this must be emitted only after
                # every DMA (on the same engine AND queue) whose data is
                # needed sooner.
                w = 1024 // pieces
                for p in range(pieces):
                    col = c * 1024 + p * w
                    kt0 = 8 * c + p * (w // 128)
                    eng.dma_start_transpose(
                        out=vaug3[:, kt0:kt0 + w // 128, 0:64],
                        in_=kvt_sb[0:64, col:col + w],
                    )

            def emit_q_chunk(c):
                # Q^T lands on partitions 64:128 so the QK matmul can take
                # K^T straight from kvt's lower half (matching base partition)
                for blk in (2 * c, 2 * c + 1):
                    qp = projps.tile([128, 512], F32, tag="proj")
                    for j in range(4):
                        nc.tensor.matmul(
                            qp[64:128, :],
                            lhsT=wpack_sb[:, 512 + j * H:512 + (j + 1) * H],
                            rhs=xtq_sb[j][:, blk * 512:(blk + 1) * 512],
                            start=(j == 0),
                            stop=(j == 3),
                        )
                    nc.vector.tensor_scalar_add(
                        qtp_sb[64:128, blk * 512:(blk + 1) * 512],
                        qp[64:128, :], bq_ap
                    )

            def emit_slot(s):
                nkt = NKT[s]
                ngrp = nkt // 2
                otp = otps.tile([H + 1, 512], F32, tag="otp")
                pts = {}
                for g in range(ngrp + 1):
                    if g < ngrp:
                        stp = stps.tile([128, 1024], F32, tag="stp")
                        for u in range(2):
                            kt = 2 * g + u
                            nc.tensor.matmul(
                                stp[:, u * 512:(u + 1) * 512],
                                lhsT=kvt_sb[64:128, kt * 128:(kt + 1) * 128],
                                rhs=qtp_sb[64:128, s * 512:(s + 1) * 512],
                                start=True,
                                stop=True,
                            )
                        pt = ptp.tile([128, 1024], BF16, tag="pt")
                        nc.scalar.activation(
                            pt[:], stp[:], mybir.ActivationFunctionType.Exp,
                            scale=1.0 / 64.0,
                        )
                        if g >= ngrp - 4:
                            gm = g - (ngrp - 4)
                            nc.vector.tensor_mul(
                                pt[:],
                                pt[:],
                                mask_sb[:, s, gm * 1024:(gm + 1) * 1024],
                            )
                        pts[g] = pt
                    if g >= 1:
                        ptm = pts.pop(g - 1)
                        for u in range(2):
                            kt = 2 * (g - 1) + u
                            nc.tensor.matmul(
                                otp[:],
                                lhsT=vaug3[:, kt, 0:65],
                                rhs=ptm[:, u * 512:(u + 1) * 512],
                                start=(kt == 0),
                                stop=(kt == nkt - 1),
                            )
                # epilogue A: stash ot (incl. denominator row 64) in bf16;
                # dnrow is a partition-0 copy of the denominator row for the
                # K=1 transpose matmuls
                ot_sb = epi.tile([H + 1, 512], BF16, tag="ot_sb")
                dnrow = epi.tile([1, 512], BF16, tag="dnrow")
                # for the final slot these copies are the kernel tail; the
                # scalar engine has finished its exps by then and is idle
                if s == 3:
                    nc.vector.tensor_copy(ot_sb[:], otp[:])
                    nc.scalar.copy(dnrow[:], otp[H:H + 1, :])
                else:
                    nc.vector.tensor_copy(ot_sb[:], otp[:])
                    nc.vector.tensor_copy(dnrow[:], otp[H:H + 1, :])
                return ot_sb, dnrow

            deferred_outs = []

            def emit_epi_b(s, ot_sb, dnrow):
                # the final slot's epilogue is the kernel tail: fan its scale
                # and output DMA across two engines/queues each
                last = s == 3
                for t in range(4):
                    dnp = projps.tile([128, 1], F32, tag="proj")
                    nc.tensor.matmul(
                        dnp[:],
                        lhsT=dnrow[:, t * 128:(t + 1) * 128],
                        rhs=ones_sb[:],
                        start=True,
                        stop=True,
                    )
                    recip = epi.tile([128, 1], F32, tag="recip")
                    nc.vector.reciprocal(recip[:], dnp[:])
                    yp = projps.tile([128, 512], F32, tag="proj")
                    nc.tensor.matmul(
                        yp[:],
                        lhsT=ot_sb[:, t * 128:(t + 1) * 128],
                        rhs=wo_sb[:],
                        start=True,
                        stop=True,
                    )
                    ysb = epi.tile([128, 512], BF16, tag="ysb", bufs=14)
                    if last and t % 2 == 1:
                        nc.scalar.activation(
                            ysb[:], yp[:], mybir.ActivationFunctionType.Copy,
                            scale=recip[:],
                        )
                    else:
                        nc.vector.tensor_scalar_mul(ysb[:], yp[:], recip[:])
                    if last:
                        out_eng = nc.scalar if t % 2 == 1 else nc.sync
                        out_eng.dma_start(
                            out=out_d[s * 512 + t * 128:s * 512 + (t + 1) * 128, :],
                            in_=ysb[:],
                        )
                    else:
                        # defer the out DMA: issuing it now would block the
                        # sync engine on the ysb semaphore while load DMAs
                        # still need issuing
                        deferred_outs.append((s, t, ysb))

            emit_kv_chunk(0)
            emit_xt_rest(1024, 2048)
            emit_xt_rest(2048, 4096)
            # slot 0's V transpose on scalar: its engine-block window ends
            # exactly when slot 0's first exp could start anyway
            emit_vtr(0, nc.scalar)
            emit_warm(3)
            emit_q_chunk(0)
            emit_warm(2)
            emit_mask(0)
            emit_mask(1)
            ot0 = emit_slot(0)
            emit_kv_chunk(1)
            emit_xtq_rest()
            emit_vtr(1, nc.sync)
            emit_mask(2)
            emit_mask(3)
            emit_epi_b(0, *ot0)
            ot1 = emit_slot(1)
            emit_kv_chunk(2)
            emit_q_chunk(1)
            emit_vtr(2, nc.sync)
            ot2 = emit_slot(2)
            emit_kv_chunk(3)
            emit_vtr(3, nc.sync)
            # epilogues 1+2 run here: slot 3's first 12 groups need no
            # vector work, so the bunched recip/scale ops cannot stall it
            emit_epi_b(1, *ot1)
            emit_epi_b(2, *ot2)
            for s, t, ysb in deferred_outs:
                nc.sync.dma_start(
                    out=out_d[s * 512 + t * 128:s * 512 + (t + 1) * 128, :],
                    in_=ysb[:],
                )
            deferred_outs.clear()
            ot3 = emit_slot(3)
            emit_epi_b(3, *ot3)

    nc.compile()
    return nc


_NC_CACHE = {}


def _make_in_maps(x, Wq, bq, Wk, bk, Wv, bv, Wo, bo):
    wkv = np.concatenate([Wv, Wk], axis=1)                    # (512, 128)
    wo_aug = np.concatenate([Wo, (bv @ Wo + bo)[None, :]], axis=0).astype(ml_dtypes.bfloat16)
    # wpack[p, j*128+m] = wkv[j*128+p, m]; wpack[p, 512+j*64+h] = Wq[j*128+p, h]
    wpack = np.zeros((128, 768), np.float32)
    for j in range(4):
        wpack[:, j * 128:(j + 1) * 128] = wkv[j * 128:(j + 1) * 128, :]
        wpack[:, 512 + j * H:512 + (j + 1) * H] = Wq[j * 128:(j + 1) * 128, :]
    wpack = wpack.astype(ml_dtypes.bfloat16)

    in_maps = []
    for c in range(8):
        b = c // 2
        blocks = BLOCKS_EVEN if c % 2 == 0 else BLOCKS_ODD
        xt = np.ascontiguousarray(x[b].T).astype(ml_dtypes.bfloat16)  # (512, 4096)
        qcols = np.concatenate(
            [np.arange(blk * QB, (blk + 1) * QB) for blk in blocks]
        )
        xtq = np.ascontiguousarray(xt[:, qcols])               # (512, 2048)
        fpack = np.zeros((128, 6), np.float32)
        fpack[64:, 0] = bk
        fpack[0:64, 1] = bq
        for s in range(4):
            fpack[:, 2 + s] = (NKT[s] - 8) * 128 - 512 * blocks[s]
        in_maps.append({
            "xt": xt,
            "xtq": xtq,
            "wpack": wpack,
            "wo": wo_aug,
            "fpack": fpack,
        })
    return in_maps


def kernel(x, Wq, bq, Wk, bk, Wv, bv, Wo, bo):
    global LAST_EXEC_TIME_NS, LAST_RESULTS
    x = np.asarray(x, dtype=np.float32)
    Wq, bq = np.asarray(Wq, np.float32), np.asarray(bq, np.float32)
    Wk, bk = np.asarray(Wk, np.float32), np.asarray(bk, np.float32)
    Wv, bv = np.asarray(Wv, np.float32), np.asarray(bv, np.float32)
    Wo, bo = np.asarray(Wo, np.float32), np.asarray(bo, np.float32)

    if "nc" not in _NC_CACHE:
        _NC_CACHE["nc"] = _build_nc()
    nc = _NC_CACHE["nc"]

    in_maps = _make_in_maps(x, Wq, bq, Wk, bk, Wv, bv, Wo, bo)

    trace = os.environ.get("KERNEL_TRACE", "1") == "1"
    if trace:
        trace = _install_ntff_hook()
    tmpdir = os.environ.get("KERNEL_TRACE_DIR") or None
    try:
        res = run_bass_kernel_spmd(
            nc, in_maps, core_ids=list(range(8)), trace=trace, tmpdir=tmpdir
        )
    except Exception:
        if not trace:
            raise
        res = run_bass_kernel_spmd(nc, in_maps, core_ids=list(range(8)), trace=False)
    LAST_EXEC_TIME_NS = res.exec_time_ns
    LAST_RESULTS = res

    out = np.empty((B, S, D), np.float32)
    for c in range(8):
        b = c // 2
        blocks = BLOCKS_EVEN if c % 2 == 0 else BLOCKS_ODD
        shard = res.results[c]["out"].astype(np.float32)
        for sidx, blk in enumerate(blocks):
            out[b, blk * QB:(blk + 1) * QB, :] = shard[sidx * QB:(sidx + 1) * QB, :]
    return out
